# revision 28
# baseline (speedup 1.0000x reference)
"""Trainium2 Bass kernel for the CCA module (attention + 1x1 convs + BN/ReLU).

Contract: kernel(**inputs) takes the FULL fp32 inputs (shapes hardcoded below),
shards the batch over 8 NeuronCores (2 samples each), runs a Bass/Tile kernel
via run_bass_kernel_spmd, and returns the FULL (16, 512, 64, 64) fp32 output.

Host-side preprocessing (numpy):
  - BN (eval mode) folded into the 1x1 conv weights/biases.
  - attT (pixel-partitioned att) precomputed on host — no device DMA transpose.
  - att2: att with the two pixel halves stacked into 128 partitions (enables
    row-group-concurrent K=64 matmuls on both PE halves).
  - Activations bf16 (PE streams bf16 @ 1 col/cycle; fp32 accumulate in PSUM).
  - y returned bf16, upcast on host.

Device-side per sample s (C=512, C8=64, HW=4096 pixels):
  projT[n,k] = sum_c x[c,n] * key_w[k,c]      (x tiles stationary)
  energy[k,q] = sum_n projT[n,k] * attT[n,q]  (accumulated per x-quarter)
  attn = softmax_q(energy)                    (max/exp/sum on ACT+DVE)
  w1aT_q = attn^T @ [w1T|w1T], duplicated into both partition halves
  out2_full = relu(W1a @ att + b1) as [128, HW/2]: both pixel halves at once
              via two row-group-concurrent matmuls per 512-col block
  y[o,n] = relu(sum_c W2b[o,c] x[c,n] + sum_k W2a[o,k] out2[k,n] + b2[o])
           (x-part K=128 chains; out2-part K=64 matmuls run pairwise
            concurrent in opposite PE row halves)

Schedule: PE warmed up with dummy matmuls while sample-0 inputs stream in;
c2 x-only partial sums for 4 PSUM banks are prefilled pre-attention; sample-1
attention work is interleaved into sample-0's c2 to hide evac latencies.
"""

from contextlib import ExitStack

import numpy as np

import concourse.bacc as bacc
import concourse.tile as tile
from concourse import mybir
from concourse.bass_utils import run_bass_kernel_spmd

N_CORES = 8
B, C, H, W = 16, 512, 64, 64
C8 = C // 8          # 64
HW = H * W           # 4096
S = B // N_CORES     # samples per core = 2
NCH = C // 128       # channel chunks = 4
NT = HW // 128       # 128-wide pixel tiles = 32
NJ = HW // 512       # 512-wide pixel blocks = 8
EPS = 1e-5
NWARM = 0            # warmup matmuls: disabled — extra PE duty triggers the
                     # P0 power downclock (2.4 -> 2.0 GHz), a far worse trade
NFILL = 0            # HAM keep-alive fillers per load-phase slot (same risk)

BF16 = mybir.dt.bfloat16
F32 = mybir.dt.float32
NP_BF16 = mybir.dt.np(BF16)

_BUILT = None
PHASE_MARKS = []  # (label, n_insts_at_mark) for trace attribution


def _mark(nc, label):
    PHASE_MARKS.append((label, len(nc.inst_map)))


class _Ctx:
    """Bag of state shared by the emission helpers."""
    pass


def _emit_loads(k, s, staged=False, extra_after_q0=None):
    """Issue sample-s input DMAs. DMA *issue* on the Sync engine costs
    ~0.7us per instruction, so keep the count low; for sample 0 stage the
    wire order so projT/energy can start as early as possible."""
    nc = k.nc
    _mark(nc, f"loads_{s}")
    x_sb = k.xpool.tile([128, NCH, HW], BF16, name=f"x_sb_{s}", tag="x")
    attT = k.attTpool.tile([128, NT, C8], BF16, name=f"attT_{s}", tag="attT")
    att2 = k.att2pool.tile([128, HW // 2], BF16, name=f"att2_{s}", tag="att2")
    nq = HW // 4

    def xpart(a, b):
        nc.sync.dma_start(out=x_sb[:, :, a * nq:b * nq],
                          in_=k.dram["x"][s][:, :, a * nq:b * nq])

    if staged:
        # wire: xq0 | kb | xq1 | attT | x-half-1 | att2
        xpart(0, 1)
        if extra_after_q0 is not None:
            extra_after_q0()
        xpart(1, 2)
        nc.sync.dma_start(out=attT, in_=k.dram["attT"][s])
        xpart(2, 4)
    else:
        xpart(0, 2)
        nc.sync.dma_start(out=attT, in_=k.dram["attT"][s])
        xpart(2, 4)
    nc.sync.dma_start(out=att2, in_=k.dram["att2"][s])
    return x_sb, attT, att2


def _emit_projT_quarter(k, s, q, x_sb):
    """projT for pixel quarter q (8 n-tiles); returns the evacuated bf16 tile."""
    nc = k.nc
    _mark(nc, f"projT_{s}_{q}")
    pA = k.psA.tile([128, 8, C8], F32, name=f"pA_{s}_{q}", tag="pa")
    for i in range(8):
        nt = q * 8 + i
        for ci in range(NCH):
            nc.tensor.matmul(
                pA[:, i, :],
                lhsT=x_sb[:, ci, nt * 128:(nt + 1) * 128],
                rhs=k.sb["kwT"][:, ci, :],
                start=(ci == 0), stop=(ci == NCH - 1))
    pj = k.projTpool.tile([128, 8, C8], BF16, name=f"pj_{s}_{q}", tag="pj",
                          bufs=8)
    nc.vector.tensor_add(pj, pA, k.sb["kb_bc"])
    return pj


def _emit_energy_quarter(k, s, q, pj, attT, ps_e):
    nc = k.nc
    _mark(nc, f"energy_{s}_{q}")
    for i in range(8):
        nt = q * 8 + i
        nc.tensor.matmul(ps_e, lhsT=pj[:, i, :], rhs=attT[:, nt, :],
                         start=(q == 0 and i == 0), stop=(q == 3 and i == 7))


def _emit_c2x_chain(k, s, ps, ot, j, x_sb, start=True):
    """The 4-chunk x-only accumulation for c2 block (ot, j) into psum ps."""
    nc = k.nc
    for ci in range(NCH):
        nc.tensor.matmul(
            ps,
            lhsT=k.sb["w2bT"][:, ci, ot * 128:(ot + 1) * 128],
            rhs=x_sb[:, ci, j * 512:(j + 1) * 512],
            start=(start and ci == 0), stop=False)


def _emit_c2x_pair(k, s, ps_l, ps_h, ot, jl, jh, x_sb):
    """x-only chains for blocks jl (lo) and jh (hi), sharing each LDWEIGHTS."""
    nc = k.nc
    for ci in range(NCH):
        nc.tensor.matmul(
            ps_l, lhsT=k.sb["w2bT"][:, ci, ot * 128:(ot + 1) * 128],
            rhs=x_sb[:, ci, jl * 512:(jl + 1) * 512],
            start=(ci == 0), stop=False)
        nc.tensor.matmul(
            ps_h, lhsT=k.sb["w2bT"][:, ci, ot * 128:(ot + 1) * 128],
            rhs=x_sb[:, ci, jh * 512:(jh + 1) * 512],
            start=(ci == 0), stop=False)


def _emit_c2o_mm(k, ps, ot, j, out2_full, hi):
    """out2-part matmul for block j; hi selects the upper PE row half."""
    nc = k.nc
    if hi:
        nc.tensor.matmul(
            ps, lhsT=k.sb["wa_blob"][64:128, ot * 128:(ot + 1) * 128],
            rhs=out2_full[64:128, (j - 4) * 512:(j - 3) * 512],
            start=False, stop=True)
    else:
        nc.tensor.matmul(
            ps, lhsT=k.sb["wa_blob"][0:64, ot * 128:(ot + 1) * 128],
            rhs=out2_full[0:64, j * 512:(j + 1) * 512],
            start=False, stop=True)


def _emit_y_evac(k, s, ps, ot, j, y_sb, use_act):
    """PSUM -> y_sb bf16 with +b2 and relu; alternate ACT/DVE."""
    nc = k.nc
    dst = y_sb[:, j * 512:(j + 1) * 512]
    b2col = k.sb["bias_blob"][:, 1 + ot:2 + ot]
    if use_act:
        nc.scalar.activation(dst, ps, mybir.ActivationFunctionType.Relu,
                             bias=b2col, scale=1.0)
    else:
        nc.vector.tensor_scalar(out=dst, in0=ps,
                                scalar1=b2col, scalar2=0.0,
                                op0=mybir.AluOpType.add,
                                op1=mybir.AluOpType.max)


def _emit_attention_tail(k, s, ps_e, att2):
    """softmax + W1a + out2_full for sample s (after energy accumulation)."""
    nc = k.nc
    _mark(nc, f"softmax_{s}")
    negmax = k.small.tile([C8, 1], F32, name=f"negmax_{s}")
    nc.vector.tensor_reduce(negmax, ps_e, axis=mybir.AxisListType.X,
                            op=mybir.AluOpType.max, negate=True)
    attn_exp = k.small.tile([C8, C8], F32, name=f"attn_exp_{s}")
    sumexp = k.small.tile([C8, 1], F32, name=f"sumexp_{s}")
    nc.scalar.activation(attn_exp, ps_e, mybir.ActivationFunctionType.Exp,
                         bias=negmax, scale=1.0, accum_out=sumexp)
    rec = k.small.tile([C8, 1], F32, name=f"rec_{s}")
    nc.vector.reciprocal(rec, sumexp)
    attn_dup = k.small.tile([C8, 128], BF16, name=f"attn_dup_{s}")
    nc.vector.tensor_scalar_mul(attn_dup[:, 0:C8], attn_exp, rec)
    nc.vector.tensor_scalar_mul(attn_dup[:, C8:128], attn_exp, rec)

    # w1aT duplicated along both output cols and partition halves in one MM
    ps_w2 = k.psB.tile([128, 128], F32, name=f"ps_w2_{s}", tag="sm")
    nc.tensor.matmul(ps_w2, lhsT=attn_dup, rhs=k.sb["wa_blob"][0:64, 512:640],
                     start=True, stop=True)
    w1aT_q = k.small.tile([128, 128], BF16, name=f"w1aT_q_{s}")
    nc.scalar.copy(w1aT_q, ps_w2)

    _mark(nc, f"out2_{s}")
    out2_full = k.out2pool.tile([128, HW // 2], BF16, name=f"out2_{s}",
                                tag="out2")
    for jb in range(4):
        sl = slice(jb * 512, (jb + 1) * 512)
        ps_lo = k.psB.tile([128, 512], F32, name=f"ps_o_lo_{s}_{jb}", tag="sm")
        ps_hi = k.psB.tile([128, 512], F32, name=f"ps_o_hi_{s}_{jb}", tag="sm")
        nc.tensor.matmul(ps_lo, lhsT=w1aT_q[0:64, :], rhs=att2[0:64, sl],
                         start=True, stop=True)
        nc.tensor.matmul(ps_hi, lhsT=w1aT_q[64:128, :], rhs=att2[64:128, sl],
                         start=True, stop=True)
        nc.scalar.activation(out2_full[0:64, sl], ps_lo[0:64, :],
                             mybir.ActivationFunctionType.Relu,
                             bias=k.sb["bias_blob"][0:64, 0:1], scale=1.0)
        nc.vector.tensor_scalar(out=out2_full[64:128, sl], in0=ps_hi[64:128, :],
                                scalar1=k.sb["bias_blob"][64:128, 0:1],
                                scalar2=0.0,
                                op0=mybir.AluOpType.add,
                                op1=mybir.AluOpType.max)
    return out2_full


def _emit_store_full(k, s, ot, y_sb, split=1):
    # HWDGE (sync) only: SWDGE stores put descriptor-ring traffic on the
    # SBUF AXI ports and slow the PE's rhs streaming. One store per output
    # tile: DMA *issue* costs ~0.7us of Sync-queue time apiece.
    nc = k.nc
    step = HW // split
    for i in range(split):
        a = i * step
        nc.sync.dma_start(out=k.dram["y"][s, ot, :, a:a + step],
                          in_=y_sb[:, a:a + step])


def _emit_filler(k, n):
    """HAM keep-alive: dummy matmuls so the PE clock gate stays open while
    the engine is data-starved during the load phase."""
    nc = k.nc
    ps = k.psA.tile([128, 64], F32, name="filler_ps", tag="pa")
    for _ in range(n):
        nc.tensor.matmul(ps, lhsT=k.sb["warm"], rhs=k.sb["warm"][:, 0:64],
                         start=True, stop=True)


def _get_y(k, s, ot):
    key = (s, ot)
    if key not in k.ytiles:
        k.ytiles[key] = k.ypool.tile([128, HW], BF16, name=f"y_sb_{s}_{ot}",
                                     tag="y", bufs=5)
    return k.ytiles[key]


def _maybe_store(k, s, ot, y_sb, last):
    done = k.jdone[(s, ot)]
    if last:
        # quarter-granularity: store each 1024-col span as soon as both of
        # its j-blocks are evacuated, so the final drain starts early
        nc = k.nc
        for qt in range(4):
            ready = (2 * qt in done) and (2 * qt + 1 in done)
            if ready and qt not in k.stored.setdefault((s, ot), set()):
                k.stored[(s, ot)].add(qt)
                a = qt * 1024
                nc.sync.dma_start(out=k.dram["y"][s, ot, :, a:a + 1024],
                                  in_=y_sb[:, a:a + 1024])
    elif len(done) == 8 and (s, ot) not in k.stored:
        k.stored[(s, ot)] = True
        _emit_store_full(k, s, ot, y_sb, split=1)


def _emit_c2_ot(k, s, ot, x_sb, out2_full, j0_done, last, j4_done=False):
    """Full c2 for output tile ot: x chains + row-paired out2 MMs + evacs."""
    nc = k.nc
    _mark(nc, f"c2_{s}_{ot}")
    y_sb = _get_y(k, s, ot)
    done = k.jdone.setdefault((s, ot), [])
    if j0_done:
        pairs = [(1, 5), (2, 6), (3, 7)]
        singles = [] if j4_done else [4]
    else:
        pairs = [(0, 4), (1, 5), (2, 6), (3, 7)]
        singles = []
    for jl, jh in pairs:
        ps_l = k.psC.tile([128, 512], F32, name=f"ps_y_{s}_{ot}_{jl}", tag="c2")
        ps_h = k.psC.tile([128, 512], F32, name=f"ps_y_{s}_{ot}_{jh}", tag="c2")
        _emit_c2x_pair(k, s, ps_l, ps_h, ot, jl, jh, x_sb)
        _emit_c2o_mm(k, ps_l, ot, jl, out2_full, hi=False)
        _emit_c2o_mm(k, ps_h, ot, jh, out2_full, hi=True)
        _emit_y_evac(k, s, ps_l, ot, jl, y_sb, use_act=(jl % 2 == 0))
        _emit_y_evac(k, s, ps_h, ot, jh, y_sb, use_act=(jh % 2 == 0))
        done += [jl, jh]
        _maybe_store(k, s, ot, y_sb, last)
    for j in singles:
        ps = k.psC.tile([128, 512], F32, name=f"ps_y_{s}_{ot}_{j}", tag="c2")
        _emit_c2x_chain(k, s, ps, ot, j, x_sb, start=True)
        _emit_c2o_mm(k, ps, ot, j, out2_full, hi=(j >= 4))
        _emit_y_evac(k, s, ps, ot, j, y_sb, use_act=(j % 2 == 0))
        done.append(j)
        _maybe_store(k, s, ot, y_sb, last)


def _build():
    """Build and finalize the per-core Bass program (same on all 8 cores)."""
    nc = bacc.Bacc("TRN2", target_bir_lowering=False, debug=False)

    k = _Ctx()
    k.nc = nc
    k.stored = {}
    k.jdone = {}
    k.ytiles = {}
    k.dram = {
        "x": nc.dram_tensor("x", [S, 128, NCH, HW], BF16, kind="ExternalInput"),
        "attT": nc.dram_tensor("attT", [S, 128, NT, C8], BF16,
                               kind="ExternalInput"),
        "att2": nc.dram_tensor("att2", [S, 128, HW // 2], BF16,
                               kind="ExternalInput"),
        "kwT": nc.dram_tensor("kwT", [128, NCH, C8], BF16,
                              kind="ExternalInput"),
        "kb_bc": nc.dram_tensor("kb_bc", [128, 8, C8], F32,
                                kind="ExternalInput"),
        "wa_blob": nc.dram_tensor("wa_blob", [128, 640], BF16,
                                  kind="ExternalInput"),
        "bias_blob": nc.dram_tensor("bias_blob", [128, 5], F32,
                                    kind="ExternalInput"),
        "w2bT": nc.dram_tensor("w2bT", [128, NCH, C], BF16,
                               kind="ExternalInput"),
        "y": nc.dram_tensor("y", [S, 4, 128, HW], BF16, kind="ExternalOutput"),
    }

    with nc.allow_low_precision("bf16 activations; fp32 accumulate in PSUM"), \
         tile.TileContext(nc) as tc:
        with ExitStack() as ctx:
            k.consts = ctx.enter_context(tc.tile_pool(name="consts", bufs=1))
            k.xpool = ctx.enter_context(tc.tile_pool(name="xpool", bufs=2))
            k.att2pool = ctx.enter_context(tc.tile_pool(name="att2pool", bufs=2))
            k.attTpool = ctx.enter_context(tc.tile_pool(name="attTpool", bufs=2))
            k.projTpool = ctx.enter_context(tc.tile_pool(name="projTpool",
                                                         bufs=2))
            k.out2pool = ctx.enter_context(tc.tile_pool(name="out2pool", bufs=2))
            k.ypool = ctx.enter_context(tc.tile_pool(name="ypool", bufs=2))
            k.small = ctx.enter_context(tc.tile_pool(name="small", bufs=2))
            k.psA = ctx.enter_context(tc.tile_pool(name="psA", bufs=2,
                                                   space="PSUM"))
            k.psB = ctx.enter_context(tc.tile_pool(name="psB", bufs=2,
                                                   space="PSUM"))
            k.psC = ctx.enter_context(tc.tile_pool(name="psC", bufs=4,
                                                   space="PSUM"))
            k.sb = {}

            const_specs = {
                "kwT": ([128, NCH, C8], BF16),
                "kb_bc": ([128, 8, C8], F32),
                "wa_blob": ([128, 640], BF16),
                "bias_blob": ([128, 5], F32),
                "w2bT": ([128, NCH, C], BF16),
            }

            def load_consts(names):
                for name in names:
                    shape, dt = const_specs[name]
                    t = k.consts.tile(shape, dt, name=f"{name}_sb")
                    nc.sync.dma_start(out=t, in_=k.dram[name][:])
                    k.sb[name] = t

            # ---- PE warmup: optional dummy matmuls while sample-0 inputs
            # stream in. NOTE: extra PE duty cycle can tip the chip into the
            # P0 power state (PE 2.4 -> 2.0 GHz), costing far more than the
            # HAM clock gate it avoids — keep this minimal or zero. --------
            _mark(nc, "warmup")
            warm_sb = k.consts.tile([128, 128], BF16, name="warm_sb")
            nc.vector.memset(warm_sb, 0.01)
            k.sb["warm"] = warm_sb
            if NWARM:
                warm_ps = k.psA.tile([128, 128], F32, name="warm_ps", tag="pa")
                for i in range(NWARM):
                    nc.tensor.matmul(warm_ps, lhsT=warm_sb, rhs=warm_sb,
                                     start=True, stop=True)
                warm_out = k.consts.tile([128, 128], BF16, name="warm_out")
                nc.scalar.copy(warm_out, warm_ps)

            # ---- sample 0 loads. Wire order is the critical path: kwT and
            # w2bT (needed by projT/prefill) lead, then x staged with attT,
            # then att2 and the small attention-tail weights. --------------
            load_consts(["kwT"])
            x0, attT0, att20 = _emit_loads(
                k, 0, staged=True,
                extra_after_q0=lambda: load_consts(["kb_bc", "w2bT"]))
            load_consts(["bias_blob", "wa_blob"])

            # ---- sample 0: projT + energy per quarter, c2-x prefill ------
            ps_e0 = k.psB.tile([C8, C8], F32, name="ps_e_0", tag="sm")
            prefill = {}
            for q in range(4):
                pj = _emit_projT_quarter(k, 0, q, x0)
                if q == 0:
                    _mark(nc, "prefill_a")
                    for ot in (0, 1):
                        ps = k.psC.tile([128, 512], F32,
                                        name=f"ps_y_0_{ot}_0", tag="c2")
                        _emit_c2x_chain(k, 0, ps, ot, 0, x0, start=True)
                        prefill[ot] = ps
                _emit_energy_quarter(k, 0, q, pj, attT0, ps_e0)
                if q == 1:
                    _mark(nc, "prefill_b")
                    for ot in (2, 3):
                        ps = k.psC.tile([128, 512], F32,
                                        name=f"ps_y_0_{ot}_0", tag="c2")
                        _emit_c2x_chain(k, 0, ps, ot, 0, x0, start=True)
                        prefill[ot] = ps
                if q < 3 and NFILL:
                    _emit_filler(k, NFILL)

            # ---- sample 1 loads go on the wire behind sample 0's --------
            x1, attT1, att21 = _emit_loads(k, 1)

            # two more c2-x chains (j4 of ot0/ot1) into the now-free psA
            # banks: fills the PE while the attention tail's softmax/evac
            # chain runs, and keeps the HAM clock gate open
            _mark(nc, "prefill_j4")
            prefill2 = {}
            for ot in (0, 1):
                ps = k.psA.tile([128, 512], F32, name=f"ps_y_0_{ot}_4",
                                tag="pa")
                _emit_c2x_chain(k, 0, ps, ot, 4, x0, start=True)
                prefill2[ot] = ps

            # ---- sample 0 attention tail + c2, interleaved with sample 1's
            # projT/energy so PSUM evac latencies hide under c2 matmuls ----
            out20 = _emit_attention_tail(k, 0, ps_e0, att20)

            # finish ALL prefilled blocks first: frees their PSUM banks
            # before any c2 pair rotates onto them (avoids a FIFO deadlock
            # between the PE queue and pool rotation). The (ot,j0)-lo and
            # (ot,j4)-hi finishing matmuls pair into opposite row halves.
            _mark(nc, "prefill_fin")
            for ot in range(4):
                _emit_c2o_mm(k, prefill[ot], ot, 0, out20, hi=False)
                if ot in prefill2:
                    _emit_c2o_mm(k, prefill2[ot], ot, 4, out20, hi=True)
            for ot in range(4):
                _emit_y_evac(k, 0, prefill[ot], ot, 0, _get_y(k, 0, ot),
                             use_act=(ot % 2 == 0))
                k.jdone.setdefault((0, ot), []).append(0)
                if ot in prefill2:
                    _emit_y_evac(k, 0, prefill2[ot], ot, 4, _get_y(k, 0, ot),
                                 use_act=(ot % 2 == 1))
                    k.jdone[(0, ot)].append(4)

            _emit_c2_ot(k, 0, 0, x0, out20, j0_done=True, last=False,
                        j4_done=True)

            ps_e1 = k.psB.tile([C8, C8], F32, name="ps_e_1", tag="sm")
            pj1 = {}
            for q in (0, 1):
                pj1[q] = _emit_projT_quarter(k, 1, q, x1)
                _emit_energy_quarter(k, 1, q, pj1[q], attT1, ps_e1)

            _emit_c2_ot(k, 0, 1, x0, out20, j0_done=True, last=False,
                        j4_done=True)

            for q in (2, 3):
                pj1[q] = _emit_projT_quarter(k, 1, q, x1)
                _emit_energy_quarter(k, 1, q, pj1[q], attT1, ps_e1)

            _emit_c2_ot(k, 0, 2, x0, out20, j0_done=True, last=False)

            out21 = _emit_attention_tail(k, 1, ps_e1, att21)

            _emit_c2_ot(k, 0, 3, x0, out20, j0_done=True, last=False)

            for ot in range(4):
                _emit_c2_ot(k, 1, ot, x1, out21, j0_done=False, last=(ot == 3))

    nc.finalize()
    return nc


def _get_built():
    global _BUILT
    if _BUILT is None:
        _BUILT = _build()
    return _BUILT


def _prep_weights(key_w, key_b, c1_w, c1_b, c1_gamma, c1_beta, c1_mean, c1_var,
                  c2_w, c2_b, c2_gamma, c2_beta, c2_mean, c2_var):
    s1 = c1_gamma / np.sqrt(c1_var + EPS)
    w1 = c1_w * s1[:, None]                       # (64, 64)
    b1 = c1_b * s1 + c1_beta - c1_mean * s1       # (64,)
    s2 = c2_gamma / np.sqrt(c2_var + EPS)
    w2 = c2_w * s2[:, None]                       # (512, 576)
    b2 = c2_b * s2 + c2_beta - c2_mean * s2       # (512,)
    w2a = w2[:, :C8]                              # (512, 64)  applies to out2
    w2b = w2[:, C8:]                              # (512, 512) applies to x

    w2aT = np.ascontiguousarray(w2a.T)            # (64, 512)
    w1T = np.ascontiguousarray(w1.T)              # (64, 64)

    # wa_blob [128, 640]: cols 0:512 = w2aT duplicated into both partition
    # halves; cols 512:640 = [w1T | w1T] on partitions 0:64 (junk elsewhere).
    wa_blob = np.zeros((128, 640), np.float32)
    wa_blob[0:64, 0:512] = w2aT
    wa_blob[64:128, 0:512] = w2aT
    wa_blob[0:64, 512:576] = w1T
    wa_blob[0:64, 576:640] = w1T
    # bias_blob [128, 5]: col 0 = b1 duplicated; cols 1:5 = b2 as (4,128).T
    bias_blob = np.zeros((128, 5), np.float32)
    bias_blob[:, 0] = np.concatenate([b1, b1])
    bias_blob[:, 1:5] = b2.reshape(4, 128).T
    return {
        "kwT": np.ascontiguousarray(
            key_w.T.reshape(NCH, 128, C8).transpose(1, 0, 2)).astype(NP_BF16),
        "kb_bc": np.ascontiguousarray(
            np.broadcast_to(key_b[None, None, :], (128, 8, C8))).astype(np.float32),
        "wa_blob": wa_blob.astype(NP_BF16),
        "bias_blob": bias_blob.astype(np.float32),
        "w2bT": np.ascontiguousarray(
            w2b.T.reshape(NCH, 128, C).transpose(1, 0, 2)).astype(NP_BF16),
    }


def _prep_in_maps(inputs):
    x = np.asarray(inputs["x"], np.float32).reshape(B, C, HW)
    att = np.asarray(inputs["att"], np.float32).reshape(B, C8, HW)
    weights = _prep_weights(**{kk: np.asarray(v, np.float32)
                               for kk, v in inputs.items()
                               if kk not in ("x", "att")})
    in_maps = []
    for c in range(N_CORES):
        s0 = c * S
        x_core = np.ascontiguousarray(
            x[s0:s0 + S].reshape(S, NCH, 128, HW).transpose(0, 2, 1, 3)
        ).astype(NP_BF16)
        att_c = att[s0:s0 + S]                       # (S, 64, HW)
        # attT[s, p, nt, q] = att[s, q, nt*128 + p]
        attT_core = np.ascontiguousarray(
            att_c.reshape(S, C8, NT, 128).transpose(0, 3, 2, 1)
        ).astype(NP_BF16)
        # att2[s, 0:64, n] = att[s, :, n]; att2[s, 64:128, n] = att[s, :, 2048+n]
        att2_core = np.ascontiguousarray(
            att_c.reshape(S, C8, 2, HW // 2).transpose(0, 2, 1, 3)
            .reshape(S, 128, HW // 2)).astype(NP_BF16)
        m = {"x": x_core, "attT": attT_core, "att2": att2_core}
        m.update(weights)
        in_maps.append(m)
    return in_maps


def kernel(**inputs):
    nc = _get_built()
    in_maps = _prep_in_maps(inputs)
    res = run_bass_kernel_spmd(nc, in_maps, core_ids=list(range(N_CORES)))
    y = np.concatenate([np.asarray(res.results[c]["y"], dtype=np.float32)
                        for c in range(N_CORES)], axis=0)
    return np.ascontiguousarray(y.reshape(B, C, H, W)).astype(np.float32)


# revision 31
# speedup vs baseline: 1.1774x; 1.1774x over previous
"""Trainium2 Bass kernel for the CCA module (attention + 1x1 convs + BN/ReLU).

Contract: kernel(**inputs) takes the FULL fp32 inputs (shapes hardcoded below),
shards the batch over 8 NeuronCores (2 samples each), runs a Bass/Tile kernel
via run_bass_kernel_spmd, and returns the FULL (16, 512, 64, 64) fp32 output.

Host-side preprocessing (numpy):
  - BN (eval mode) folded into the 1x1 conv weights/biases.
  - attT (pixel-partitioned att) precomputed on host — no device DMA transpose.
  - att2: att with the two pixel halves stacked into 128 partitions (enables
    row-group-concurrent K=64 matmuls on both PE halves).
  - Activations bf16 (PE streams bf16 @ 1 col/cycle; fp32 accumulate in PSUM).
  - y returned bf16, upcast on host.

Device-side per sample s (C=512, C8=64, HW=4096 pixels):
  projT[n,k] = sum_c x[c,n] * key_w[k,c]      (x tiles stationary)
  energy[k,q] = sum_n projT[n,k] * attT[n,q]  (accumulated per x-quarter)
  attn = softmax_q(energy)                    (max/exp/sum on ACT+DVE)
  w1aT_q = attn^T @ [w1T|w1T], duplicated into both partition halves
  out2_full = relu(W1a @ att + b1) as [128, HW/2]: both pixel halves at once
              via two row-group-concurrent matmuls per 512-col block
  y[o,n] = relu(sum_c W2b[o,c] x[c,n] + sum_k W2a[o,k] out2[k,n] + b2[o])
           (x-part K=128 chains; out2-part K=64 matmuls run pairwise
            concurrent in opposite PE row halves)

Schedule: PE warmed up with dummy matmuls while sample-0 inputs stream in;
c2 x-only partial sums for 4 PSUM banks are prefilled pre-attention; sample-1
attention work is interleaved into sample-0's c2 to hide evac latencies.
"""

from contextlib import ExitStack

import numpy as np

import concourse.bacc as bacc
import concourse.tile as tile
from concourse import mybir
from concourse.bass_utils import run_bass_kernel_spmd

N_CORES = 8
B, C, H, W = 16, 512, 64, 64
C8 = C // 8          # 64
HW = H * W           # 4096
S = B // N_CORES     # samples per core = 2
NCH = C // 128       # channel chunks = 4
NT = HW // 128       # 128-wide pixel tiles = 32
NJ = HW // 512       # 512-wide pixel blocks = 8
EPS = 1e-5
NWARM = 0            # warmup matmuls: disabled — extra PE duty triggers the
                     # P0 power downclock (2.4 -> 2.0 GHz), a far worse trade
NFILL = 0            # HAM keep-alive fillers per load-phase slot (same risk)

BF16 = mybir.dt.bfloat16
F32 = mybir.dt.float32
NP_BF16 = mybir.dt.np(BF16)

_BUILT = None
PHASE_MARKS = []  # (label, n_insts_at_mark) for trace attribution


def _mark(nc, label):
    PHASE_MARKS.append((label, len(nc.inst_map)))


class _Ctx:
    """Bag of state shared by the emission helpers."""
    pass


def _emit_loads(k, s, staged=False, extra_after_q0=None):
    """Issue sample-s input DMAs. DMA *issue* on the Sync engine costs
    ~0.7us per instruction, so keep the count low; for sample 0 stage the
    wire order so projT/energy can start as early as possible."""
    nc = k.nc
    _mark(nc, f"loads_{s}")
    x_sb = k.xpool.tile([128, NCH, HW], BF16, name=f"x_sb_{s}", tag="x")
    attT = k.attTpool.tile([128, NT, C8], BF16, name=f"attT_{s}", tag="attT")
    att2 = k.att2pool.tile([128, HW // 2], BF16, name=f"att2_{s}", tag="att2")
    nq = HW // 4

    def xpart(a, b):
        nc.sync.dma_start(out=x_sb[:, :, a * nq:b * nq],
                          in_=k.dram["x"][s][:, :, a * nq:b * nq])

    if staged:
        # wire: xq0 | kb | xq1 | attT | x-half-1 | att2
        xpart(0, 1)
        if extra_after_q0 is not None:
            extra_after_q0()
        xpart(1, 2)
        nc.sync.dma_start(out=attT, in_=k.dram["attT"][s])
        xpart(2, 4)
    else:
        xpart(0, 2)
        nc.sync.dma_start(out=attT, in_=k.dram["attT"][s])
        xpart(2, 4)
    nc.sync.dma_start(out=att2, in_=k.dram["att2"][s])
    return x_sb, attT, att2


def _emit_projT_quarter(k, s, q, x_sb):
    """projT for pixel quarter q (8 n-tiles); returns the evacuated bf16 tile."""
    nc = k.nc
    _mark(nc, f"projT_{s}_{q}")
    pA = k.psA.tile([128, 8, C8], F32, name=f"pA_{s}_{q}", tag="pa")
    for i in range(8):
        nt = q * 8 + i
        for ci in range(NCH):
            nc.tensor.matmul(
                pA[:, i, :],
                lhsT=x_sb[:, ci, nt * 128:(nt + 1) * 128],
                rhs=k.sb["kwT"][:, ci, :],
                start=(ci == 0), stop=(ci == NCH - 1))
    pj = k.projTpool.tile([128, 8, C8], BF16, name=f"pj_{s}_{q}", tag="pj",
                          bufs=8)
    nc.vector.tensor_add(pj, pA, k.sb["kb_bc"])
    return pj


def _emit_energy_quarter(k, s, q, pj, attT, ps_e):
    nc = k.nc
    _mark(nc, f"energy_{s}_{q}")
    for i in range(8):
        nt = q * 8 + i
        nc.tensor.matmul(ps_e, lhsT=pj[:, i, :], rhs=attT[:, nt, :],
                         start=(q == 0 and i == 0), stop=(q == 3 and i == 7))


def _emit_c2x_chain(k, s, ps, ot, j, x_sb, start=True):
    """The 4-chunk x-only accumulation for c2 block (ot, j) into psum ps."""
    nc = k.nc
    for ci in range(NCH):
        nc.tensor.matmul(
            ps,
            lhsT=k.sb["w2bT"][:, ci, ot * 128:(ot + 1) * 128],
            rhs=x_sb[:, ci, j * 512:(j + 1) * 512],
            start=(start and ci == 0), stop=False)


def _emit_c2x_pair(k, s, ps_l, ps_h, ot, jl, jh, x_sb):
    """x-only chains for blocks jl (lo) and jh (hi), sharing each LDWEIGHTS."""
    nc = k.nc
    for ci in range(NCH):
        nc.tensor.matmul(
            ps_l, lhsT=k.sb["w2bT"][:, ci, ot * 128:(ot + 1) * 128],
            rhs=x_sb[:, ci, jl * 512:(jl + 1) * 512],
            start=(ci == 0), stop=False)
        nc.tensor.matmul(
            ps_h, lhsT=k.sb["w2bT"][:, ci, ot * 128:(ot + 1) * 128],
            rhs=x_sb[:, ci, jh * 512:(jh + 1) * 512],
            start=(ci == 0), stop=False)


def _emit_c2o_mm(k, ps, ot, j, out2_full, hi):
    """out2-part matmul for block j; hi selects the upper PE row half."""
    nc = k.nc
    if hi:
        nc.tensor.matmul(
            ps, lhsT=k.sb["wa_blob"][64:128, ot * 128:(ot + 1) * 128],
            rhs=out2_full[64:128, (j - 4) * 512:(j - 3) * 512],
            start=False, stop=True)
    else:
        nc.tensor.matmul(
            ps, lhsT=k.sb["wa_blob"][0:64, ot * 128:(ot + 1) * 128],
            rhs=out2_full[0:64, j * 512:(j + 1) * 512],
            start=False, stop=True)


def _emit_y_evac(k, s, ps, ot, j, y_sb, use_act):
    """PSUM -> y_sb bf16 with +b2 and relu; alternate ACT/DVE."""
    nc = k.nc
    dst = y_sb[:, j * 512:(j + 1) * 512]
    b2col = k.sb["bias_blob"][:, 1 + ot:2 + ot]
    if use_act:
        nc.scalar.activation(dst, ps, mybir.ActivationFunctionType.Relu,
                             bias=b2col, scale=1.0)
    else:
        nc.vector.tensor_scalar(out=dst, in0=ps,
                                scalar1=b2col, scalar2=0.0,
                                op0=mybir.AluOpType.add,
                                op1=mybir.AluOpType.max)


def _emit_attention_tail(k, s, ps_e, att2):
    """softmax + W1a + out2_full for sample s (after energy accumulation)."""
    nc = k.nc
    _mark(nc, f"softmax_{s}")
    negmax = k.small.tile([C8, 1], F32, name=f"negmax_{s}")
    nc.vector.tensor_reduce(negmax, ps_e, axis=mybir.AxisListType.X,
                            op=mybir.AluOpType.max, negate=True)
    attn_exp = k.small.tile([C8, C8], F32, name=f"attn_exp_{s}")
    sumexp = k.small.tile([C8, 1], F32, name=f"sumexp_{s}")
    nc.scalar.activation(attn_exp, ps_e, mybir.ActivationFunctionType.Exp,
                         bias=negmax, scale=1.0, accum_out=sumexp)
    rec = k.small.tile([C8, 1], F32, name=f"rec_{s}")
    nc.vector.reciprocal(rec, sumexp)
    attn_dup = k.small.tile([C8, 128], BF16, name=f"attn_dup_{s}")
    nc.vector.tensor_scalar_mul(attn_dup[:, 0:C8], attn_exp, rec)
    nc.vector.tensor_scalar_mul(attn_dup[:, C8:128], attn_exp, rec)

    # w1aT duplicated along both output cols and partition halves in one MM
    ps_w2 = k.psB.tile([128, 128], F32, name=f"ps_w2_{s}", tag="sm")
    nc.tensor.matmul(ps_w2, lhsT=attn_dup, rhs=k.sb["wa_blob"][0:64, 512:640],
                     start=True, stop=True)
    w1aT_q = k.small.tile([128, 128], BF16, name=f"w1aT_q_{s}")
    nc.scalar.copy(w1aT_q, ps_w2)

    _mark(nc, f"out2_{s}")
    out2_full = k.out2pool.tile([128, HW // 2], BF16, name=f"out2_{s}",
                                tag="out2")
    for jb in range(4):
        sl = slice(jb * 512, (jb + 1) * 512)
        ps_lo = k.psB.tile([128, 512], F32, name=f"ps_o_lo_{s}_{jb}", tag="sm")
        ps_hi = k.psB.tile([128, 512], F32, name=f"ps_o_hi_{s}_{jb}", tag="sm")
        nc.tensor.matmul(ps_lo, lhsT=w1aT_q[0:64, :], rhs=att2[0:64, sl],
                         start=True, stop=True)
        nc.tensor.matmul(ps_hi, lhsT=w1aT_q[64:128, :], rhs=att2[64:128, sl],
                         start=True, stop=True)
        nc.scalar.activation(out2_full[0:64, sl], ps_lo[0:64, :],
                             mybir.ActivationFunctionType.Relu,
                             bias=k.sb["bias_blob"][0:64, 0:1], scale=1.0)
        nc.vector.tensor_scalar(out=out2_full[64:128, sl], in0=ps_hi[64:128, :],
                                scalar1=k.sb["bias_blob"][64:128, 0:1],
                                scalar2=0.0,
                                op0=mybir.AluOpType.add,
                                op1=mybir.AluOpType.max)
    return out2_full


def _emit_store_full(k, s, ot, y_sb, split=1):
    # HWDGE (sync) only: SWDGE stores put descriptor-ring traffic on the
    # SBUF AXI ports and slow the PE's rhs streaming. One store per output
    # tile: DMA *issue* costs ~0.7us of Sync-queue time apiece.
    nc = k.nc
    step = HW // split
    for i in range(split):
        a = i * step
        nc.sync.dma_start(out=k.dram["y"][s, ot, :, a:a + step],
                          in_=y_sb[:, a:a + step])


def _emit_filler(k, n):
    """HAM keep-alive: dummy matmuls so the PE clock gate stays open while
    the engine is data-starved during the load phase."""
    nc = k.nc
    ps = k.psA.tile([128, 64], F32, name="filler_ps", tag="pa")
    for _ in range(n):
        nc.tensor.matmul(ps, lhsT=k.sb["warm"], rhs=k.sb["warm"][:, 0:64],
                         start=True, stop=True)


def _get_y(k, s, ot):
    key = (s, ot)
    if key not in k.ytiles:
        k.ytiles[key] = k.ypool.tile([128, HW], BF16, name=f"y_sb_{s}_{ot}",
                                     tag="y", bufs=5)
    return k.ytiles[key]


def _maybe_store(k, s, ot, y_sb, last, quarters=False):
    done = k.jdone[(s, ot)]
    if last or quarters:
        # quarter-granularity: store each 1024-col span as soon as both of
        # its j-blocks are evacuated, so the final drain starts early
        nc = k.nc
        for qt in range(4):
            ready = (2 * qt in done) and (2 * qt + 1 in done)
            if ready and qt not in k.stored.setdefault((s, ot), set()):
                k.stored[(s, ot)].add(qt)
                a = qt * 1024
                nc.sync.dma_start(out=k.dram["y"][s, ot, :, a:a + 1024],
                                  in_=y_sb[:, a:a + 1024])
    elif len(done) == 8 and (s, ot) not in k.stored:
        k.stored[(s, ot)] = True
        _emit_store_full(k, s, ot, y_sb, split=1)


def _emit_c2_ot(k, s, ot, x_sb, out2_full, j0_done, last, j4_done=False,
                wide_psum=False, quarters=False):
    """Full c2 for output tile ot: x chains + row-paired out2 MMs + evacs.

    wide_psum: alternate pairs between psC and psA/psB (only safe once
    projT/energy/out2 are all done with those pools, i.e. sample 1's c2) —
    doubles the pair pipeline depth to 8 banks."""
    nc = k.nc
    _mark(nc, f"c2_{s}_{ot}")
    y_sb = _get_y(k, s, ot)
    done = k.jdone.setdefault((s, ot), [])
    if j0_done:
        pairs = [(1, 5), (2, 6), (3, 7)]
        singles = [] if j4_done else [4]
    else:
        pairs = [(0, 4), (1, 5), (2, 6), (3, 7)]
        singles = []
    for pi, (jl, jh) in enumerate(pairs):
        if wide_psum and pi % 2 == 1:
            ps_l = k.psA.tile([128, 512], F32, name=f"ps_y_{s}_{ot}_{jl}",
                              tag="pa")
            ps_h = k.psB.tile([128, 512], F32, name=f"ps_y_{s}_{ot}_{jh}",
                              tag="sm")
        else:
            ps_l = k.psC.tile([128, 512], F32, name=f"ps_y_{s}_{ot}_{jl}",
                              tag="c2")
            ps_h = k.psC.tile([128, 512], F32, name=f"ps_y_{s}_{ot}_{jh}",
                              tag="c2")
        _emit_c2x_pair(k, s, ps_l, ps_h, ot, jl, jh, x_sb)
        _emit_c2o_mm(k, ps_l, ot, jl, out2_full, hi=False)
        _emit_c2o_mm(k, ps_h, ot, jh, out2_full, hi=True)
        _emit_y_evac(k, s, ps_l, ot, jl, y_sb, use_act=(jl % 2 == 0))
        _emit_y_evac(k, s, ps_h, ot, jh, y_sb, use_act=(jh % 2 == 0))
        done += [jl, jh]
        _maybe_store(k, s, ot, y_sb, last, quarters)
    for j in singles:
        ps = k.psC.tile([128, 512], F32, name=f"ps_y_{s}_{ot}_{j}", tag="c2")
        _emit_c2x_chain(k, s, ps, ot, j, x_sb, start=True)
        _emit_c2o_mm(k, ps, ot, j, out2_full, hi=(j >= 4))
        _emit_y_evac(k, s, ps, ot, j, y_sb, use_act=(j % 2 == 0))
        done.append(j)
        _maybe_store(k, s, ot, y_sb, last, quarters)


def _build():
    """Build and finalize the per-core Bass program (same on all 8 cores)."""
    nc = bacc.Bacc("TRN2", target_bir_lowering=False, debug=False)

    k = _Ctx()
    k.nc = nc
    k.stored = {}
    k.jdone = {}
    k.ytiles = {}
    k.dram = {
        "x": nc.dram_tensor("x", [S, 128, NCH, HW], BF16, kind="ExternalInput"),
        "attT": nc.dram_tensor("attT", [S, 128, NT, C8], BF16,
                               kind="ExternalInput"),
        "att2": nc.dram_tensor("att2", [S, 128, HW // 2], BF16,
                               kind="ExternalInput"),
        "kwT": nc.dram_tensor("kwT", [128, NCH, C8], BF16,
                              kind="ExternalInput"),
        "kb_bc": nc.dram_tensor("kb_bc", [128, 8, C8], F32,
                                kind="ExternalInput"),
        "wa_blob": nc.dram_tensor("wa_blob", [128, 640], BF16,
                                  kind="ExternalInput"),
        "bias_blob": nc.dram_tensor("bias_blob", [128, 5], F32,
                                    kind="ExternalInput"),
        "w2bT": nc.dram_tensor("w2bT", [128, NCH, C], BF16,
                               kind="ExternalInput"),
        "y": nc.dram_tensor("y", [S, 4, 128, HW], BF16, kind="ExternalOutput"),
    }

    with nc.allow_low_precision("bf16 activations; fp32 accumulate in PSUM"), \
         tile.TileContext(nc) as tc:
        with ExitStack() as ctx:
            k.consts = ctx.enter_context(tc.tile_pool(name="consts", bufs=1))
            k.xpool = ctx.enter_context(tc.tile_pool(name="xpool", bufs=2))
            k.att2pool = ctx.enter_context(tc.tile_pool(name="att2pool", bufs=2))
            k.attTpool = ctx.enter_context(tc.tile_pool(name="attTpool", bufs=2))
            k.projTpool = ctx.enter_context(tc.tile_pool(name="projTpool",
                                                         bufs=2))
            k.out2pool = ctx.enter_context(tc.tile_pool(name="out2pool", bufs=2))
            k.ypool = ctx.enter_context(tc.tile_pool(name="ypool", bufs=2))
            k.small = ctx.enter_context(tc.tile_pool(name="small", bufs=2))
            k.psA = ctx.enter_context(tc.tile_pool(name="psA", bufs=2,
                                                   space="PSUM"))
            k.psB = ctx.enter_context(tc.tile_pool(name="psB", bufs=2,
                                                   space="PSUM"))
            k.psC = ctx.enter_context(tc.tile_pool(name="psC", bufs=4,
                                                   space="PSUM"))
            k.sb = {}

            const_specs = {
                "kwT": ([128, NCH, C8], BF16),
                "kb_bc": ([128, 8, C8], F32),
                "wa_blob": ([128, 640], BF16),
                "bias_blob": ([128, 5], F32),
                "w2bT": ([128, NCH, C], BF16),
            }

            def load_consts(names):
                for name in names:
                    shape, dt = const_specs[name]
                    t = k.consts.tile(shape, dt, name=f"{name}_sb")
                    nc.sync.dma_start(out=t, in_=k.dram[name][:])
                    k.sb[name] = t

            # ---- PE warmup: optional dummy matmuls while sample-0 inputs
            # stream in. NOTE: extra PE duty cycle can tip the chip into the
            # P0 power state (PE 2.4 -> 2.0 GHz), costing far more than the
            # HAM clock gate it avoids — keep this minimal or zero. --------
            _mark(nc, "warmup")
            warm_sb = k.consts.tile([128, 128], BF16, name="warm_sb")
            nc.vector.memset(warm_sb, 0.01)
            k.sb["warm"] = warm_sb
            if NWARM:
                warm_ps = k.psA.tile([128, 128], F32, name="warm_ps", tag="pa")
                for i in range(NWARM):
                    nc.tensor.matmul(warm_ps, lhsT=warm_sb, rhs=warm_sb,
                                     start=True, stop=True)
                warm_out = k.consts.tile([128, 128], BF16, name="warm_out")
                nc.scalar.copy(warm_out, warm_ps)

            # ---- sample 0 loads. Wire order is the critical path: kwT and
            # w2bT (needed by projT/prefill) lead, then x staged with attT,
            # then att2 and the small attention-tail weights. --------------
            load_consts(["kwT"])
            x0, attT0, att20 = _emit_loads(
                k, 0, staged=True,
                extra_after_q0=lambda: load_consts(["kb_bc", "w2bT"]))
            load_consts(["bias_blob", "wa_blob"])

            # ---- sample 0: projT + energy per quarter, c2-x prefill ------
            ps_e0 = k.psB.tile([C8, C8], F32, name="ps_e_0", tag="sm")
            prefill = {}
            for q in range(4):
                pj = _emit_projT_quarter(k, 0, q, x0)
                if q == 0:
                    _mark(nc, "prefill_a")
                    for ot in (0, 1):
                        ps = k.psC.tile([128, 512], F32,
                                        name=f"ps_y_0_{ot}_0", tag="c2")
                        _emit_c2x_chain(k, 0, ps, ot, 0, x0, start=True)
                        prefill[ot] = ps
                _emit_energy_quarter(k, 0, q, pj, attT0, ps_e0)
                if q == 1:
                    _mark(nc, "prefill_b")
                    for ot in (2, 3):
                        ps = k.psC.tile([128, 512], F32,
                                        name=f"ps_y_0_{ot}_0", tag="c2")
                        _emit_c2x_chain(k, 0, ps, ot, 0, x0, start=True)
                        prefill[ot] = ps
                if q < 3 and NFILL:
                    _emit_filler(k, NFILL)

            # ---- sample 1 loads go on the wire behind sample 0's --------
            x1, attT1, att21 = _emit_loads(k, 1)

            # two more c2-x chains (j4 of ot0/ot1) into the now-free psA
            # banks: fills the PE while the attention tail's softmax/evac
            # chain runs, and keeps the HAM clock gate open
            _mark(nc, "prefill_j4")
            prefill2 = {}
            for ot in (0, 1):
                ps = k.psA.tile([128, 512], F32, name=f"ps_y_0_{ot}_4",
                                tag="pa")
                _emit_c2x_chain(k, 0, ps, ot, 4, x0, start=True)
                prefill2[ot] = ps

            # ---- sample 0 attention tail + c2, interleaved with sample 1's
            # projT/energy so PSUM evac latencies hide under c2 matmuls ----
            out20 = _emit_attention_tail(k, 0, ps_e0, att20)

            # finish ALL prefilled blocks first: frees their PSUM banks
            # before any c2 pair rotates onto them (avoids a FIFO deadlock
            # between the PE queue and pool rotation). The (ot,j0)-lo and
            # (ot,j4)-hi finishing matmuls pair into opposite row halves.
            _mark(nc, "prefill_fin")
            for ot in range(4):
                _emit_c2o_mm(k, prefill[ot], ot, 0, out20, hi=False)
                if ot in prefill2:
                    _emit_c2o_mm(k, prefill2[ot], ot, 4, out20, hi=True)
            for ot in range(4):
                _emit_y_evac(k, 0, prefill[ot], ot, 0, _get_y(k, 0, ot),
                             use_act=(ot % 2 == 0))
                k.jdone.setdefault((0, ot), []).append(0)
                if ot in prefill2:
                    _emit_y_evac(k, 0, prefill2[ot], ot, 4, _get_y(k, 0, ot),
                                 use_act=(ot % 2 == 1))
                    k.jdone[(0, ot)].append(4)

            _emit_c2_ot(k, 0, 0, x0, out20, j0_done=True, last=False,
                        j4_done=True)

            ps_e1 = k.psB.tile([C8, C8], F32, name="ps_e_1", tag="sm")
            pj1 = {}
            for q in (0, 1):
                pj1[q] = _emit_projT_quarter(k, 1, q, x1)
                _emit_energy_quarter(k, 1, q, pj1[q], attT1, ps_e1)

            _emit_c2_ot(k, 0, 1, x0, out20, j0_done=True, last=False,
                        j4_done=True)

            for q in (2, 3):
                pj1[q] = _emit_projT_quarter(k, 1, q, x1)
                _emit_energy_quarter(k, 1, q, pj1[q], attT1, ps_e1)

            _emit_c2_ot(k, 0, 2, x0, out20, j0_done=True, last=False)

            out21 = _emit_attention_tail(k, 1, ps_e1, att21)

            _emit_c2_ot(k, 0, 3, x0, out20, j0_done=True, last=False)

            for ot in range(4):
                _emit_c2_ot(k, 1, ot, x1, out21, j0_done=False, last=(ot == 3),
                            wide_psum=True, quarters=(ot == 2))

    nc.finalize()
    return nc


def _get_built():
    global _BUILT
    if _BUILT is None:
        _BUILT = _build()
    return _BUILT


def _prep_weights(key_w, key_b, c1_w, c1_b, c1_gamma, c1_beta, c1_mean, c1_var,
                  c2_w, c2_b, c2_gamma, c2_beta, c2_mean, c2_var):
    s1 = c1_gamma / np.sqrt(c1_var + EPS)
    w1 = c1_w * s1[:, None]                       # (64, 64)
    b1 = c1_b * s1 + c1_beta - c1_mean * s1       # (64,)
    s2 = c2_gamma / np.sqrt(c2_var + EPS)
    w2 = c2_w * s2[:, None]                       # (512, 576)
    b2 = c2_b * s2 + c2_beta - c2_mean * s2       # (512,)
    w2a = w2[:, :C8]                              # (512, 64)  applies to out2
    w2b = w2[:, C8:]                              # (512, 512) applies to x

    w2aT = np.ascontiguousarray(w2a.T)            # (64, 512)
    w1T = np.ascontiguousarray(w1.T)              # (64, 64)

    # wa_blob [128, 640]: cols 0:512 = w2aT duplicated into both partition
    # halves; cols 512:640 = [w1T | w1T] on partitions 0:64 (junk elsewhere).
    wa_blob = np.zeros((128, 640), np.float32)
    wa_blob[0:64, 0:512] = w2aT
    wa_blob[64:128, 0:512] = w2aT
    wa_blob[0:64, 512:576] = w1T
    wa_blob[0:64, 576:640] = w1T
    # bias_blob [128, 5]: col 0 = b1 duplicated; cols 1:5 = b2 as (4,128).T
    bias_blob = np.zeros((128, 5), np.float32)
    bias_blob[:, 0] = np.concatenate([b1, b1])
    bias_blob[:, 1:5] = b2.reshape(4, 128).T
    return {
        "kwT": np.ascontiguousarray(
            key_w.T.reshape(NCH, 128, C8).transpose(1, 0, 2)).astype(NP_BF16),
        "kb_bc": np.ascontiguousarray(
            np.broadcast_to(key_b[None, None, :], (128, 8, C8))).astype(np.float32),
        "wa_blob": wa_blob.astype(NP_BF16),
        "bias_blob": bias_blob.astype(np.float32),
        "w2bT": np.ascontiguousarray(
            w2b.T.reshape(NCH, 128, C).transpose(1, 0, 2)).astype(NP_BF16),
    }


def _prep_in_maps(inputs):
    x = np.asarray(inputs["x"], np.float32).reshape(B, C, HW)
    att = np.asarray(inputs["att"], np.float32).reshape(B, C8, HW)
    weights = _prep_weights(**{kk: np.asarray(v, np.float32)
                               for kk, v in inputs.items()
                               if kk not in ("x", "att")})
    in_maps = []
    for c in range(N_CORES):
        s0 = c * S
        x_core = np.ascontiguousarray(
            x[s0:s0 + S].reshape(S, NCH, 128, HW).transpose(0, 2, 1, 3)
        ).astype(NP_BF16)
        att_c = att[s0:s0 + S]                       # (S, 64, HW)
        # attT[s, p, nt, q] = att[s, q, nt*128 + p]
        attT_core = np.ascontiguousarray(
            att_c.reshape(S, C8, NT, 128).transpose(0, 3, 2, 1)
        ).astype(NP_BF16)
        # att2[s, 0:64, n] = att[s, :, n]; att2[s, 64:128, n] = att[s, :, 2048+n]
        att2_core = np.ascontiguousarray(
            att_c.reshape(S, C8, 2, HW // 2).transpose(0, 2, 1, 3)
            .reshape(S, 128, HW // 2)).astype(NP_BF16)
        m = {"x": x_core, "attT": attT_core, "att2": att2_core}
        m.update(weights)
        in_maps.append(m)
    return in_maps


def kernel(**inputs):
    nc = _get_built()
    in_maps = _prep_in_maps(inputs)
    res = run_bass_kernel_spmd(nc, in_maps, core_ids=list(range(N_CORES)))
    y = np.concatenate([np.asarray(res.results[c]["y"], dtype=np.float32)
                        for c in range(N_CORES)], axis=0)
    return np.ascontiguousarray(y.reshape(B, C, H, W)).astype(np.float32)


# revision 33
# speedup vs baseline: 1.1913x; 1.0118x over previous
"""Trainium2 Bass kernel for the CCA module (attention + 1x1 convs + BN/ReLU).

Contract: kernel(**inputs) takes the FULL fp32 inputs (shapes hardcoded below),
shards the batch over 8 NeuronCores (2 samples each), runs a Bass/Tile kernel
via run_bass_kernel_spmd, and returns the FULL (16, 512, 64, 64) fp32 output.

Host-side preprocessing (numpy):
  - BN (eval mode) folded into the 1x1 conv weights/biases.
  - attT (pixel-partitioned att) precomputed on host — no device DMA transpose.
  - att2: att with the two pixel halves stacked into 128 partitions (enables
    row-group-concurrent K=64 matmuls on both PE halves).
  - Activations bf16 (PE streams bf16 @ 1 col/cycle; fp32 accumulate in PSUM).
  - y returned bf16, upcast on host.

Device-side per sample s (C=512, C8=64, HW=4096 pixels):
  projT[n,k] = sum_c x[c,n] * key_w[k,c]      (x tiles stationary)
  energy[k,q] = sum_n projT[n,k] * attT[n,q]  (accumulated per x-quarter)
  attn = softmax_q(energy)                    (max/exp/sum on ACT+DVE)
  w1aT_q = attn^T @ [w1T|w1T], duplicated into both partition halves
  out2_full = relu(W1a @ att + b1) as [128, HW/2]: both pixel halves at once
              via two row-group-concurrent matmuls per 512-col block
  y[o,n] = relu(sum_c W2b[o,c] x[c,n] + sum_k W2a[o,k] out2[k,n] + b2[o])
           (x-part K=128 chains; out2-part K=64 matmuls run pairwise
            concurrent in opposite PE row halves)

Schedule: PE warmed up with dummy matmuls while sample-0 inputs stream in;
c2 x-only partial sums for 4 PSUM banks are prefilled pre-attention; sample-1
attention work is interleaved into sample-0's c2 to hide evac latencies.
"""

from contextlib import ExitStack

import numpy as np

import concourse.bacc as bacc
import concourse.tile as tile
from concourse import mybir
from concourse.bass_utils import run_bass_kernel_spmd

N_CORES = 8
B, C, H, W = 16, 512, 64, 64
C8 = C // 8          # 64
HW = H * W           # 4096
S = B // N_CORES     # samples per core = 2
NCH = C // 128       # channel chunks = 4
NT = HW // 128       # 128-wide pixel tiles = 32
NJ = HW // 512       # 512-wide pixel blocks = 8
EPS = 1e-5
NWARM = 0            # warmup matmuls: disabled — extra PE duty triggers the
                     # P0 power downclock (2.4 -> 2.0 GHz), a far worse trade
NFILL = 0            # HAM keep-alive fillers per load-phase slot (same risk)

BF16 = mybir.dt.bfloat16
F32 = mybir.dt.float32
NP_BF16 = mybir.dt.np(BF16)

_BUILT = None
PHASE_MARKS = []  # (label, n_insts_at_mark) for trace attribution


def _mark(nc, label):
    PHASE_MARKS.append((label, len(nc.inst_map)))


class _Ctx:
    """Bag of state shared by the emission helpers."""
    pass


def _emit_loads(k, s, staged=False, extra_after_q0=None):
    """Issue sample-s input DMAs. DMA *issue* on the Sync engine costs
    ~0.7us per instruction, so keep the count low; for sample 0 stage the
    wire order so projT/energy can start as early as possible."""
    nc = k.nc
    _mark(nc, f"loads_{s}")
    x_sb = k.xpool.tile([128, NCH, HW], BF16, name=f"x_sb_{s}", tag="x")
    attT = k.attTpool.tile([128, NT, C8], BF16, name=f"attT_{s}", tag="attT")
    att2 = k.att2pool.tile([128, HW // 2], BF16, name=f"att2_{s}", tag="att2")
    nq = HW // 4

    def xpart(a, b):
        nc.sync.dma_start(out=x_sb[:, :, a * nq:b * nq],
                          in_=k.dram["x"][s][:, :, a * nq:b * nq])

    if staged:
        # wire: xq0 | kb,w2bT | xq1 | attT | att2 | x-half-1. att2 must not
        # trail the 2MB x-half: the out2 matmuls gate on it right after the
        # energy chain completes.
        xpart(0, 1)
        if extra_after_q0 is not None:
            extra_after_q0()
        xpart(1, 2)
        nc.sync.dma_start(out=attT, in_=k.dram["attT"][s])
        nc.sync.dma_start(out=att2, in_=k.dram["att2"][s])
        xpart(2, 4)
    else:
        xpart(0, 2)
        nc.sync.dma_start(out=attT, in_=k.dram["attT"][s])
        nc.sync.dma_start(out=att2, in_=k.dram["att2"][s])
        xpart(2, 4)
    return x_sb, attT, att2


def _emit_projT_quarter(k, s, q, x_sb):
    """projT for pixel quarter q (8 n-tiles); returns the evacuated bf16 tile."""
    nc = k.nc
    _mark(nc, f"projT_{s}_{q}")
    pA = k.psA.tile([128, 8, C8], F32, name=f"pA_{s}_{q}", tag="pa")
    for i in range(8):
        nt = q * 8 + i
        for ci in range(NCH):
            nc.tensor.matmul(
                pA[:, i, :],
                lhsT=x_sb[:, ci, nt * 128:(nt + 1) * 128],
                rhs=k.sb["kwT"][:, ci, :],
                start=(ci == 0), stop=(ci == NCH - 1))
    pj = k.projTpool.tile([128, 8, C8], BF16, name=f"pj_{s}_{q}", tag="pj",
                          bufs=8)
    nc.vector.tensor_add(pj, pA, k.sb["kb_bc"])
    return pj


def _emit_energy_quarter(k, s, q, pj, attT, ps_e):
    nc = k.nc
    _mark(nc, f"energy_{s}_{q}")
    for i in range(8):
        nt = q * 8 + i
        nc.tensor.matmul(ps_e, lhsT=pj[:, i, :], rhs=attT[:, nt, :],
                         start=(q == 0 and i == 0), stop=(q == 3 and i == 7))


def _emit_c2x_chain(k, s, ps, ot, j, x_sb, start=True):
    """The 4-chunk x-only accumulation for c2 block (ot, j) into psum ps."""
    nc = k.nc
    for ci in range(NCH):
        nc.tensor.matmul(
            ps,
            lhsT=k.sb["w2bT"][:, ci, ot * 128:(ot + 1) * 128],
            rhs=x_sb[:, ci, j * 512:(j + 1) * 512],
            start=(start and ci == 0), stop=False)


def _emit_c2x_pair(k, s, ps_l, ps_h, ot, jl, jh, x_sb):
    """x-only chains for blocks jl (lo) and jh (hi), sharing each LDWEIGHTS."""
    nc = k.nc
    for ci in range(NCH):
        nc.tensor.matmul(
            ps_l, lhsT=k.sb["w2bT"][:, ci, ot * 128:(ot + 1) * 128],
            rhs=x_sb[:, ci, jl * 512:(jl + 1) * 512],
            start=(ci == 0), stop=False)
        nc.tensor.matmul(
            ps_h, lhsT=k.sb["w2bT"][:, ci, ot * 128:(ot + 1) * 128],
            rhs=x_sb[:, ci, jh * 512:(jh + 1) * 512],
            start=(ci == 0), stop=False)


def _emit_c2o_mm(k, ps, ot, j, out2_full, hi):
    """out2-part matmul for block j; hi selects the upper PE row half."""
    nc = k.nc
    if hi:
        nc.tensor.matmul(
            ps, lhsT=k.sb["wa_blob"][64:128, ot * 128:(ot + 1) * 128],
            rhs=out2_full[64:128, (j - 4) * 512:(j - 3) * 512],
            start=False, stop=True)
    else:
        nc.tensor.matmul(
            ps, lhsT=k.sb["wa_blob"][0:64, ot * 128:(ot + 1) * 128],
            rhs=out2_full[0:64, j * 512:(j + 1) * 512],
            start=False, stop=True)


def _emit_y_evac(k, s, ps, ot, j, y_sb, use_act):
    """PSUM -> y_sb bf16 with +b2 and relu; alternate ACT/DVE."""
    nc = k.nc
    dst = y_sb[:, j * 512:(j + 1) * 512]
    b2col = k.sb["bias_blob"][:, 1 + ot:2 + ot]
    if use_act:
        nc.scalar.activation(dst, ps, mybir.ActivationFunctionType.Relu,
                             bias=b2col, scale=1.0)
    else:
        nc.vector.tensor_scalar(out=dst, in0=ps,
                                scalar1=b2col, scalar2=0.0,
                                op0=mybir.AluOpType.add,
                                op1=mybir.AluOpType.max)


def _emit_attention_tail(k, s, ps_e, att2):
    """softmax + W1a + out2_full for sample s (after energy accumulation)."""
    nc = k.nc
    _mark(nc, f"softmax_{s}")
    negmax = k.small.tile([C8, 1], F32, name=f"negmax_{s}")
    nc.vector.tensor_reduce(negmax, ps_e, axis=mybir.AxisListType.X,
                            op=mybir.AluOpType.max, negate=True)
    attn_exp = k.small.tile([C8, C8], F32, name=f"attn_exp_{s}")
    sumexp = k.small.tile([C8, 1], F32, name=f"sumexp_{s}")
    nc.scalar.activation(attn_exp, ps_e, mybir.ActivationFunctionType.Exp,
                         bias=negmax, scale=1.0, accum_out=sumexp)
    rec = k.small.tile([C8, 1], F32, name=f"rec_{s}")
    nc.vector.reciprocal(rec, sumexp)
    attn_dup = k.small.tile([C8, 128], BF16, name=f"attn_dup_{s}")
    nc.vector.tensor_scalar_mul(attn_dup[:, 0:C8], attn_exp, rec)
    nc.vector.tensor_scalar_mul(attn_dup[:, C8:128], attn_exp, rec)

    # w1aT duplicated along both output cols and partition halves in one MM
    ps_w2 = k.psB.tile([128, 128], F32, name=f"ps_w2_{s}", tag="sm")
    nc.tensor.matmul(ps_w2, lhsT=attn_dup, rhs=k.sb["wa_blob"][0:64, 512:640],
                     start=True, stop=True)
    w1aT_q = k.small.tile([128, 128], BF16, name=f"w1aT_q_{s}")
    nc.scalar.copy(w1aT_q, ps_w2)

    _mark(nc, f"out2_{s}")
    out2_full = k.out2pool.tile([128, HW // 2], BF16, name=f"out2_{s}",
                                tag="out2")
    for jb in range(4):
        sl = slice(jb * 512, (jb + 1) * 512)
        ps_lo = k.psB.tile([128, 512], F32, name=f"ps_o_lo_{s}_{jb}", tag="sm")
        ps_hi = k.psB.tile([128, 512], F32, name=f"ps_o_hi_{s}_{jb}", tag="sm")
        nc.tensor.matmul(ps_lo, lhsT=w1aT_q[0:64, :], rhs=att2[0:64, sl],
                         start=True, stop=True)
        nc.tensor.matmul(ps_hi, lhsT=w1aT_q[64:128, :], rhs=att2[64:128, sl],
                         start=True, stop=True)
        nc.scalar.activation(out2_full[0:64, sl], ps_lo[0:64, :],
                             mybir.ActivationFunctionType.Relu,
                             bias=k.sb["bias_blob"][0:64, 0:1], scale=1.0)
        nc.vector.tensor_scalar(out=out2_full[64:128, sl], in0=ps_hi[64:128, :],
                                scalar1=k.sb["bias_blob"][64:128, 0:1],
                                scalar2=0.0,
                                op0=mybir.AluOpType.add,
                                op1=mybir.AluOpType.max)
    return out2_full


def _emit_store_full(k, s, ot, y_sb, split=1):
    # HWDGE (sync) only: SWDGE stores put descriptor-ring traffic on the
    # SBUF AXI ports and slow the PE's rhs streaming. One store per output
    # tile: DMA *issue* costs ~0.7us of Sync-queue time apiece.
    nc = k.nc
    step = HW // split
    for i in range(split):
        a = i * step
        nc.sync.dma_start(out=k.dram["y"][s, ot, :, a:a + step],
                          in_=y_sb[:, a:a + step])


def _emit_filler(k, n):
    """HAM keep-alive: dummy matmuls so the PE clock gate stays open while
    the engine is data-starved during the load phase."""
    nc = k.nc
    ps = k.psA.tile([128, 64], F32, name="filler_ps", tag="pa")
    for _ in range(n):
        nc.tensor.matmul(ps, lhsT=k.sb["warm"], rhs=k.sb["warm"][:, 0:64],
                         start=True, stop=True)


def _get_y(k, s, ot):
    key = (s, ot)
    if key not in k.ytiles:
        k.ytiles[key] = k.ypool.tile([128, HW], BF16, name=f"y_sb_{s}_{ot}",
                                     tag="y", bufs=5)
    return k.ytiles[key]


def _maybe_store(k, s, ot, y_sb, last, quarters=False):
    done = k.jdone[(s, ot)]
    if last or quarters:
        # quarter-granularity: store each 1024-col span as soon as both of
        # its j-blocks are evacuated, so the final drain starts early
        nc = k.nc
        for qt in range(4):
            ready = (2 * qt in done) and (2 * qt + 1 in done)
            if ready and qt not in k.stored.setdefault((s, ot), set()):
                k.stored[(s, ot)].add(qt)
                a = qt * 1024
                nc.sync.dma_start(out=k.dram["y"][s, ot, :, a:a + 1024],
                                  in_=y_sb[:, a:a + 1024])
    elif len(done) == 8 and (s, ot) not in k.stored:
        k.stored[(s, ot)] = True
        _emit_store_full(k, s, ot, y_sb, split=1)


def _emit_c2_ot(k, s, ot, x_sb, out2_full, j0_done, last, j4_done=False,
                wide_psum=False, quarters=False):
    """Full c2 for output tile ot: x chains + row-paired out2 MMs + evacs.

    wide_psum: alternate pairs between psC and psA/psB (only safe once
    projT/energy/out2 are all done with those pools, i.e. sample 1's c2) —
    doubles the pair pipeline depth to 8 banks."""
    nc = k.nc
    _mark(nc, f"c2_{s}_{ot}")
    y_sb = _get_y(k, s, ot)
    done = k.jdone.setdefault((s, ot), [])
    if j0_done:
        pairs = [(1, 5), (2, 6), (3, 7)]
        singles = [] if j4_done else [4]
    else:
        pairs = [(0, 4), (1, 5), (2, 6), (3, 7)]
        singles = []
    for pi, (jl, jh) in enumerate(pairs):
        if wide_psum and pi % 2 == 1:
            ps_l = k.psA.tile([128, 512], F32, name=f"ps_y_{s}_{ot}_{jl}",
                              tag="pa")
            ps_h = k.psB.tile([128, 512], F32, name=f"ps_y_{s}_{ot}_{jh}",
                              tag="sm")
        else:
            ps_l = k.psC.tile([128, 512], F32, name=f"ps_y_{s}_{ot}_{jl}",
                              tag="c2")
            ps_h = k.psC.tile([128, 512], F32, name=f"ps_y_{s}_{ot}_{jh}",
                              tag="c2")
        _emit_c2x_pair(k, s, ps_l, ps_h, ot, jl, jh, x_sb)
        _emit_c2o_mm(k, ps_l, ot, jl, out2_full, hi=False)
        _emit_c2o_mm(k, ps_h, ot, jh, out2_full, hi=True)
        _emit_y_evac(k, s, ps_l, ot, jl, y_sb, use_act=(jl % 2 == 0))
        _emit_y_evac(k, s, ps_h, ot, jh, y_sb, use_act=(jh % 2 == 0))
        done += [jl, jh]
        _maybe_store(k, s, ot, y_sb, last, quarters)
    for j in singles:
        ps = k.psC.tile([128, 512], F32, name=f"ps_y_{s}_{ot}_{j}", tag="c2")
        _emit_c2x_chain(k, s, ps, ot, j, x_sb, start=True)
        _emit_c2o_mm(k, ps, ot, j, out2_full, hi=(j >= 4))
        _emit_y_evac(k, s, ps, ot, j, y_sb, use_act=(j % 2 == 0))
        done.append(j)
        _maybe_store(k, s, ot, y_sb, last, quarters)


def _build():
    """Build and finalize the per-core Bass program (same on all 8 cores)."""
    nc = bacc.Bacc("TRN2", target_bir_lowering=False, debug=False)

    k = _Ctx()
    k.nc = nc
    k.stored = {}
    k.jdone = {}
    k.ytiles = {}
    k.dram = {
        "x": nc.dram_tensor("x", [S, 128, NCH, HW], BF16, kind="ExternalInput"),
        "attT": nc.dram_tensor("attT", [S, 128, NT, C8], BF16,
                               kind="ExternalInput"),
        "att2": nc.dram_tensor("att2", [S, 128, HW // 2], BF16,
                               kind="ExternalInput"),
        "kwT": nc.dram_tensor("kwT", [128, NCH, C8], BF16,
                              kind="ExternalInput"),
        "kb_bc": nc.dram_tensor("kb_bc", [128, 8, C8], F32,
                                kind="ExternalInput"),
        "wa_blob": nc.dram_tensor("wa_blob", [128, 640], BF16,
                                  kind="ExternalInput"),
        "bias_blob": nc.dram_tensor("bias_blob", [128, 5], F32,
                                    kind="ExternalInput"),
        "w2bT": nc.dram_tensor("w2bT", [128, NCH, C], BF16,
                               kind="ExternalInput"),
        "y": nc.dram_tensor("y", [S, 4, 128, HW], BF16, kind="ExternalOutput"),
    }

    with nc.allow_low_precision("bf16 activations; fp32 accumulate in PSUM"), \
         tile.TileContext(nc) as tc:
        with ExitStack() as ctx:
            k.consts = ctx.enter_context(tc.tile_pool(name="consts", bufs=1))
            k.xpool = ctx.enter_context(tc.tile_pool(name="xpool", bufs=2))
            k.att2pool = ctx.enter_context(tc.tile_pool(name="att2pool", bufs=2))
            k.attTpool = ctx.enter_context(tc.tile_pool(name="attTpool", bufs=2))
            k.projTpool = ctx.enter_context(tc.tile_pool(name="projTpool",
                                                         bufs=2))
            k.out2pool = ctx.enter_context(tc.tile_pool(name="out2pool", bufs=2))
            k.ypool = ctx.enter_context(tc.tile_pool(name="ypool", bufs=2))
            k.small = ctx.enter_context(tc.tile_pool(name="small", bufs=2))
            k.psA = ctx.enter_context(tc.tile_pool(name="psA", bufs=2,
                                                   space="PSUM"))
            k.psB = ctx.enter_context(tc.tile_pool(name="psB", bufs=2,
                                                   space="PSUM"))
            k.psC = ctx.enter_context(tc.tile_pool(name="psC", bufs=4,
                                                   space="PSUM"))
            k.sb = {}

            const_specs = {
                "kwT": ([128, NCH, C8], BF16),
                "kb_bc": ([128, 8, C8], F32),
                "wa_blob": ([128, 640], BF16),
                "bias_blob": ([128, 5], F32),
                "w2bT": ([128, NCH, C], BF16),
            }

            def load_consts(names):
                for name in names:
                    shape, dt = const_specs[name]
                    t = k.consts.tile(shape, dt, name=f"{name}_sb")
                    nc.sync.dma_start(out=t, in_=k.dram[name][:])
                    k.sb[name] = t

            # ---- PE warmup: optional dummy matmuls while sample-0 inputs
            # stream in. NOTE: extra PE duty cycle can tip the chip into the
            # P0 power state (PE 2.4 -> 2.0 GHz), costing far more than the
            # HAM clock gate it avoids — keep this minimal or zero. --------
            _mark(nc, "warmup")
            warm_sb = k.consts.tile([128, 128], BF16, name="warm_sb")
            nc.vector.memset(warm_sb, 0.01)
            k.sb["warm"] = warm_sb
            if NWARM:
                warm_ps = k.psA.tile([128, 128], F32, name="warm_ps", tag="pa")
                for i in range(NWARM):
                    nc.tensor.matmul(warm_ps, lhsT=warm_sb, rhs=warm_sb,
                                     start=True, stop=True)
                warm_out = k.consts.tile([128, 128], BF16, name="warm_out")
                nc.scalar.copy(warm_out, warm_ps)

            # ---- sample 0 loads. Wire order is the critical path: kwT and
            # w2bT (needed by projT/prefill) lead, then x staged with attT,
            # then att2 and the small attention-tail weights. --------------
            load_consts(["kwT"])
            x0, attT0, att20 = _emit_loads(
                k, 0, staged=True,
                extra_after_q0=lambda: load_consts(["kb_bc", "w2bT"]))
            load_consts(["bias_blob", "wa_blob"])

            # ---- sample 0: projT + energy per quarter, c2-x prefill ------
            ps_e0 = k.psB.tile([C8, C8], F32, name="ps_e_0", tag="sm")
            prefill = {}
            for q in range(4):
                pj = _emit_projT_quarter(k, 0, q, x0)
                if q == 0:
                    _mark(nc, "prefill_a")
                    for ot in (0, 1):
                        ps = k.psC.tile([128, 512], F32,
                                        name=f"ps_y_0_{ot}_0", tag="c2")
                        _emit_c2x_chain(k, 0, ps, ot, 0, x0, start=True)
                        prefill[ot] = ps
                _emit_energy_quarter(k, 0, q, pj, attT0, ps_e0)
                if q == 1:
                    _mark(nc, "prefill_b")
                    for ot in (2, 3):
                        ps = k.psC.tile([128, 512], F32,
                                        name=f"ps_y_0_{ot}_0", tag="c2")
                        _emit_c2x_chain(k, 0, ps, ot, 0, x0, start=True)
                        prefill[ot] = ps
                if q < 3 and NFILL:
                    _emit_filler(k, NFILL)

            # ---- sample 1 loads go on the wire behind sample 0's --------
            x1, attT1, att21 = _emit_loads(k, 1)

            # two more c2-x chains (j4 of ot0/ot1) into the now-free psA
            # banks: fills the PE while the attention tail's softmax/evac
            # chain runs, and keeps the HAM clock gate open
            _mark(nc, "prefill_j4")
            prefill2 = {}
            for ot in (0, 1):
                ps = k.psA.tile([128, 512], F32, name=f"ps_y_0_{ot}_4",
                                tag="pa")
                _emit_c2x_chain(k, 0, ps, ot, 4, x0, start=True)
                prefill2[ot] = ps

            # ---- sample 0 attention tail + c2, interleaved with sample 1's
            # projT/energy so PSUM evac latencies hide under c2 matmuls ----
            out20 = _emit_attention_tail(k, 0, ps_e0, att20)

            # finish ALL prefilled blocks first: frees their PSUM banks
            # before any c2 pair rotates onto them (avoids a FIFO deadlock
            # between the PE queue and pool rotation). The (ot,j0)-lo and
            # (ot,j4)-hi finishing matmuls pair into opposite row halves.
            _mark(nc, "prefill_fin")
            for ot in range(4):
                _emit_c2o_mm(k, prefill[ot], ot, 0, out20, hi=False)
                if ot in prefill2:
                    _emit_c2o_mm(k, prefill2[ot], ot, 4, out20, hi=True)
            for ot in range(4):
                _emit_y_evac(k, 0, prefill[ot], ot, 0, _get_y(k, 0, ot),
                             use_act=(ot % 2 == 0))
                k.jdone.setdefault((0, ot), []).append(0)
                if ot in prefill2:
                    _emit_y_evac(k, 0, prefill2[ot], ot, 4, _get_y(k, 0, ot),
                                 use_act=(ot % 2 == 1))
                    k.jdone[(0, ot)].append(4)

            _emit_c2_ot(k, 0, 0, x0, out20, j0_done=True, last=False,
                        j4_done=True)

            ps_e1 = k.psB.tile([C8, C8], F32, name="ps_e_1", tag="sm")
            pj1 = {}
            for q in (0, 1):
                pj1[q] = _emit_projT_quarter(k, 1, q, x1)
                _emit_energy_quarter(k, 1, q, pj1[q], attT1, ps_e1)

            _emit_c2_ot(k, 0, 1, x0, out20, j0_done=True, last=False,
                        j4_done=True)

            for q in (2, 3):
                pj1[q] = _emit_projT_quarter(k, 1, q, x1)
                _emit_energy_quarter(k, 1, q, pj1[q], attT1, ps_e1)

            _emit_c2_ot(k, 0, 2, x0, out20, j0_done=True, last=False)

            out21 = _emit_attention_tail(k, 1, ps_e1, att21)

            _emit_c2_ot(k, 0, 3, x0, out20, j0_done=True, last=False,
                        wide_psum=True)

            for ot in range(4):
                _emit_c2_ot(k, 1, ot, x1, out21, j0_done=False, last=(ot == 3),
                            wide_psum=True, quarters=(ot == 2))

    nc.finalize()
    return nc


def _get_built():
    global _BUILT
    if _BUILT is None:
        _BUILT = _build()
    return _BUILT


def _prep_weights(key_w, key_b, c1_w, c1_b, c1_gamma, c1_beta, c1_mean, c1_var,
                  c2_w, c2_b, c2_gamma, c2_beta, c2_mean, c2_var):
    s1 = c1_gamma / np.sqrt(c1_var + EPS)
    w1 = c1_w * s1[:, None]                       # (64, 64)
    b1 = c1_b * s1 + c1_beta - c1_mean * s1       # (64,)
    s2 = c2_gamma / np.sqrt(c2_var + EPS)
    w2 = c2_w * s2[:, None]                       # (512, 576)
    b2 = c2_b * s2 + c2_beta - c2_mean * s2       # (512,)
    w2a = w2[:, :C8]                              # (512, 64)  applies to out2
    w2b = w2[:, C8:]                              # (512, 512) applies to x

    w2aT = np.ascontiguousarray(w2a.T)            # (64, 512)
    w1T = np.ascontiguousarray(w1.T)              # (64, 64)

    # wa_blob [128, 640]: cols 0:512 = w2aT duplicated into both partition
    # halves; cols 512:640 = [w1T | w1T] on partitions 0:64 (junk elsewhere).
    wa_blob = np.zeros((128, 640), np.float32)
    wa_blob[0:64, 0:512] = w2aT
    wa_blob[64:128, 0:512] = w2aT
    wa_blob[0:64, 512:576] = w1T
    wa_blob[0:64, 576:640] = w1T
    # bias_blob [128, 5]: col 0 = b1 duplicated; cols 1:5 = b2 as (4,128).T
    bias_blob = np.zeros((128, 5), np.float32)
    bias_blob[:, 0] = np.concatenate([b1, b1])
    bias_blob[:, 1:5] = b2.reshape(4, 128).T
    return {
        "kwT": np.ascontiguousarray(
            key_w.T.reshape(NCH, 128, C8).transpose(1, 0, 2)).astype(NP_BF16),
        "kb_bc": np.ascontiguousarray(
            np.broadcast_to(key_b[None, None, :], (128, 8, C8))).astype(np.float32),
        "wa_blob": wa_blob.astype(NP_BF16),
        "bias_blob": bias_blob.astype(np.float32),
        "w2bT": np.ascontiguousarray(
            w2b.T.reshape(NCH, 128, C).transpose(1, 0, 2)).astype(NP_BF16),
    }


def _prep_in_maps(inputs):
    x = np.asarray(inputs["x"], np.float32).reshape(B, C, HW)
    att = np.asarray(inputs["att"], np.float32).reshape(B, C8, HW)
    weights = _prep_weights(**{kk: np.asarray(v, np.float32)
                               for kk, v in inputs.items()
                               if kk not in ("x", "att")})
    in_maps = []
    for c in range(N_CORES):
        s0 = c * S
        x_core = np.ascontiguousarray(
            x[s0:s0 + S].reshape(S, NCH, 128, HW).transpose(0, 2, 1, 3)
        ).astype(NP_BF16)
        att_c = att[s0:s0 + S]                       # (S, 64, HW)
        # attT[s, p, nt, q] = att[s, q, nt*128 + p]
        attT_core = np.ascontiguousarray(
            att_c.reshape(S, C8, NT, 128).transpose(0, 3, 2, 1)
        ).astype(NP_BF16)
        # att2[s, 0:64, n] = att[s, :, n]; att2[s, 64:128, n] = att[s, :, 2048+n]
        att2_core = np.ascontiguousarray(
            att_c.reshape(S, C8, 2, HW // 2).transpose(0, 2, 1, 3)
            .reshape(S, 128, HW // 2)).astype(NP_BF16)
        m = {"x": x_core, "attT": attT_core, "att2": att2_core}
        m.update(weights)
        in_maps.append(m)
    return in_maps


def kernel(**inputs):
    nc = _get_built()
    in_maps = _prep_in_maps(inputs)
    res = run_bass_kernel_spmd(nc, in_maps, core_ids=list(range(N_CORES)))
    y = np.concatenate([np.asarray(res.results[c]["y"], dtype=np.float32)
                        for c in range(N_CORES)], axis=0)
    return np.ascontiguousarray(y.reshape(B, C, H, W)).astype(np.float32)
